# revision 6
# baseline (speedup 1.0000x reference)
"""CTC batch loss kernel for Trainium2 (8 NeuronCores, batch-parallel).

Math: reference computes logp = log_softmax(log(y+eps)) = log(y+eps) - log(rowsum),
then a log-space forward DP over the extended label sequence (S = 2L+1 = 129).
We run the DP in probability space with periodic renormalization:
  loss[b] = sum_t log(rowsum[b,t]) - sum_j log(c_j[b]) - log(tail[b])
where c_j are the renorm constants and tail is the (scaled) final alpha mass in
the two valid end states.

Per-core layout (32 samples/core):
  - y_pred is transposed on host to [b, thalf, c(part), cchunk, t] so the
    per-sample gather matmul (contract over C) needs no on-chip transpose.
  - One-hot matrix O_b [1024, 130] per sample (host-built, bf16):
      lanes 0..63   = onehot(label_j)               -> u_label
      lanes 64..127 = skip_j * onehot(label_j)      -> u_label pre-masked for skips
      lane  128     = onehot(blank=1023)            -> u_blank
      lane  129     = ones                          -> rowsum
  - PE: emis[t, lane] = sum_c Y^T[c, t] * O[c, lane], accumulated over 8 c-chunks.
  - ACT copies PSUM->SBUF (bf16), then an SBUF->SBUF DMA repacks per-sample
    [128t, 130] blocks into the DP layout emis[32b, 128t, 130lane].
  - DVE runs the sequential DP: even (blank) states [32,65] and odd (label)
    states [32,64], 6 ops/step, renorm by max every 16 steps.
"""

import os
import sys
from contextlib import ExitStack

import numpy as np

sys.path.insert(0, "/opt/trn_rl_repo")
sys.path.insert(0, "/root/.axon_site/_ro/trn_rl_repo")

import ml_dtypes  # noqa: E402

B, T, C, L = 256, 256, 1024, 64
NCORES = 8
BS = B // NCORES  # 32 samples per core
S = 2 * L + 1  # 129 extended states
NLANE = 130  # 64 ul | 64 su | blank | ones
KCH = C // 128  # 8 contraction chunks
TH = 2  # t halves of 128
THW = T // TH  # 128
NORM_EVERY = 16
NNORM = T // NORM_EVERY  # 16
BLANK = C - 1


# ---------------------------------------------------------------- host prep

def host_prep_y(y_pred: np.ndarray) -> np.ndarray:
    """[B, T, C] f32 -> [B, TH, 128(c part), KCH, THW(t)] f32 contiguous."""
    yt = y_pred.reshape(B, TH, THW, KCH, 128).transpose(0, 1, 4, 3, 2)
    return np.ascontiguousarray(yt)


def host_prep_oh(y_true: np.ndarray) -> np.ndarray:
    """[B, L] int -> one-hot+aux matrix [B, 128(c part), KCH, NLANE] bf16."""
    lab = y_true.astype(np.int64)  # [B, L]
    oh = np.zeros((B, C, NLANE), dtype=np.float32)
    bidx = np.arange(B)[:, None]
    jidx = np.arange(L)[None, :]
    # u_label lanes
    oh[bidx, lab, jidx] = 1.0
    # su lanes: skip_j = (j>=1) & (lab_j != lab_{j-1})
    skip = np.zeros((B, L), dtype=np.float32)
    skip[:, 1:] = (lab[:, 1:] != lab[:, :-1]).astype(np.float32)
    oh[bidx, lab, jidx + L] = skip
    # blank lane
    oh[:, BLANK, 2 * L] = 1.0
    # ones lane (rowsum)
    oh[:, :, 2 * L + 1] = 1.0
    # [B, C, NLANE] -> [B, KCH, 128, NLANE] -> [B, 128, KCH, NLANE]
    oh = oh.reshape(B, KCH, 128, NLANE).transpose(0, 2, 1, 3)
    return np.ascontiguousarray(oh).astype(ml_dtypes.bfloat16)


# ---------------------------------------------------------------- bass build

def build_nc():
    import concourse.bass as bass
    import concourse.tile as tile
    from concourse import bacc, mybir

    f32 = mybir.dt.float32
    bf16 = mybir.dt.bfloat16

    nc = bacc.Bacc(None, target_bir_lowering=False)

    yt_d = nc.declare_dram_parameter("yt", [BS, TH, 128, KCH, THW], f32, isOutput=False)
    oh_d = nc.declare_dram_parameter("oh", [BS, 128, KCH, NLANE], bf16, isOutput=False)
    out_d = nc.declare_dram_parameter("out", [BS, 1], f32, isOutput=True)

    with tile.TileContext(nc) as tc:
        with ExitStack() as ctx:
            ohp = ctx.enter_context(tc.tile_pool(name="ohp", bufs=1))
            yp = ctx.enter_context(tc.tile_pool(name="yp", bufs=10))
            psp = ctx.enter_context(
                tc.tile_pool(name="psp", bufs=4, space=bass.MemorySpace.PSUM)
            )
            stp = ctx.enter_context(tc.tile_pool(name="stp", bufs=4))
            emp = ctx.enter_context(tc.tile_pool(name="emp", bufs=2))
            alp = ctx.enter_context(tc.tile_pool(name="alp", bufs=1))
            dpp = ctx.enter_context(tc.tile_pool(name="dpp", bufs=3))
            fin = ctx.enter_context(tc.tile_pool(name="fin", bufs=1))

            # one-hot matrices: resident for the whole kernel
            oh_sb = []
            for b in range(BS):
                t_oh = ohp.tile([128, KCH, NLANE], bf16, tag=f"oh{b}", name=f"oh{b}")
                nc.sync.dma_start(t_oh[:], oh_d[b])
                oh_sb.append(t_oh)

            # persistent DP state (double buffered even/odd alpha vectors)
            AE = [alp.tile([BS, L + 2], f32, tag=f"ae{i}", name=f"ae{i}") for i in range(2)]
            AO = [alp.tile([BS, L + 1], f32, tag=f"ao{i}", name=f"ao{i}") for i in range(2)]
            RS = fin.tile([BS, T], bf16)  # rowsums
            NRM = fin.tile([BS, NNORM], f32)  # renorm constants
            for t_ in AE + AO:
                nc.vector.memset(t_[:], 0.0)

            def dp_step(t, em, ubh):
                """One DP step: read alpha buffers (t-1)%2, write t%2."""
                tt = t % THW
                cur, prv = t % 2, (t - 1) % 2
                aep, aop = AE[prv], AO[prv]
                aen, aon = AE[cur], AO[cur]
                ul = em[:, tt, 0:L]
                su = em[:, tt, L : 2 * L]
                ub = ubh[:, tt : tt + 1]
                q = dpp.tile([BS, L], f32, tag="q")
                x1 = dpp.tile([BS, L], f32, tag="x1")
                x2 = dpp.tile([BS, L], f32, tag="x2")
                e1 = dpp.tile([BS, L + 1], f32, tag="e1")
                # odd (label) states j=0..63:
                #   ao_new[j] = (ao[j] + ae[j]) * ul[j] + ao[j-1] * su[j]
                nc.vector.tensor_add(q[:], aop[:, 1 : L + 1], aep[:, 1 : L + 1])
                nc.vector.tensor_mul(x1[:], q[:], ul)
                nc.vector.tensor_mul(x2[:], aop[:, 0:L], su)
                nc.vector.tensor_add(aon[:, 1 : L + 1], x1[:], x2[:])
                # even (blank) states j=0..64: ae_new[j] = (ae[j] + ao[j-1]) * ub
                nc.vector.tensor_add(e1[:], aep[:, 1 : L + 2], aop[:, 0 : L + 1])
                nc.vector.tensor_scalar_mul(aen[:, 1 : L + 2], e1[:], ub)
                if t % NORM_EVERY == NORM_EVERY - 1:
                    r = t // NORM_EVERY
                    m1 = dpp.tile([BS, 1], f32, tag="m1")
                    rc = dpp.tile([BS, 1], f32, tag="rc")
                    nc.vector.tensor_reduce(
                        m1[:], aen[:, 1 : L + 2], mybir.AxisListType.X,
                        mybir.AluOpType.max,
                    )
                    nc.vector.tensor_reduce(
                        NRM[:, r : r + 1], aon[:, 1 : L + 1], mybir.AxisListType.X,
                        mybir.AluOpType.max,
                    )
                    nc.vector.tensor_max(
                        NRM[:, r : r + 1], NRM[:, r : r + 1], m1[:]
                    )
                    nc.vector.reciprocal(rc[:], NRM[:, r : r + 1])
                    nc.vector.tensor_scalar_mul(
                        aen[:, 1 : L + 2], aen[:, 1 : L + 2], rc[:]
                    )
                    nc.vector.tensor_scalar_mul(
                        aon[:, 1 : L + 1], aon[:, 1 : L + 1], rc[:]
                    )

            for th in range(TH):
                em = emp.tile([BS, THW, NLANE], bf16, tag="em")
                for b in range(BS):
                    ybf = yp.tile([128, KCH, THW], bf16, tag="ybf")
                    nc.gpsimd.dma_start(ybf[:], yt_d[b, th])  # f32 -> bf16 cast DMA
                    ps = psp.tile([THW, NLANE], f32, tag="ps")
                    for k in range(KCH):
                        nc.tensor.matmul(
                            ps[:],
                            ybf[:, k, :],
                            oh_sb[b][:, k, :],
                            start=(k == 0),
                            stop=(k == KCH - 1),
                        )
                    st = stp.tile([THW, NLANE], bf16, tag="st")
                    nc.scalar.copy(st[:], ps[:])
                    # repack [128t, 130] -> emis[b, :, :] (one partition row)
                    nc.sync.dma_start(em[b : b + 1], st[:])

                # rowsum lanes out before the buffer is recycled
                nc.vector.tensor_copy(
                    RS[:, th * THW : (th + 1) * THW], em[:, :, NLANE - 1]
                )
                # u_blank lane in fp32 (tensor_scalar needs an fp32 scalar AP)
                ubh = dpp.tile([BS, THW], f32, tag="ubh", bufs=2)
                nc.vector.tensor_copy(ubh[:], em[:, :, 2 * L])

                t0 = th * THW
                if th == 0:
                    # init: alpha0 = emissions of states s=0 (blank), s=1 (label 0)
                    nc.vector.tensor_copy(AE[0][:, 1:2], em[:, 0, 2 * L : 2 * L + 1])
                    nc.vector.tensor_copy(AO[0][:, 1:2], em[:, 0, 0:1])
                    steps = range(1, THW)
                else:
                    steps = range(t0, t0 + THW)
                for t in steps:
                    dp_step(t, em, ubh)

            # ---- final: loss = sum_t log rs - sum_r log c_r - log(tail)
            last = (T - 1) % 2
            tail = fin.tile([BS, 1], f32)
            nc.vector.tensor_add(
                tail[:], AO[last][:, L : L + 1], AE[last][:, L + 1 : L + 2]
            )
            ln_tail = fin.tile([BS, 1], f32)
            nc.scalar.activation(
                ln_tail[:], tail[:], mybir.ActivationFunctionType.Ln
            )
            scr_n = fin.tile([BS, NNORM], f32)
            acc_n = fin.tile([BS, 1], f32)
            nc.scalar.activation(
                scr_n[:], NRM[:], mybir.ActivationFunctionType.Ln, accum_out=acc_n[:]
            )
            scr_r = fin.tile([BS, T], f32)
            acc_r = fin.tile([BS, 1], f32)
            nc.scalar.activation(
                scr_r[:], RS[:], mybir.ActivationFunctionType.Ln, accum_out=acc_r[:]
            )
            loss = fin.tile([BS, 1], f32)
            nc.vector.tensor_sub(loss[:], acc_r[:], acc_n[:])
            nc.vector.tensor_sub(loss[:], loss[:], ln_tail[:])
            nc.sync.dma_start(out_d[:], loss[:])

    nc.compile()
    return nc


_NC_CACHE = {}


def _get_nc():
    if "nc" not in _NC_CACHE:
        _NC_CACHE["nc"] = build_nc()
    return _NC_CACHE["nc"]


# ---------------------------------------------------------------- entrypoint

def kernel(y_true: np.ndarray, y_pred: np.ndarray, _trace: bool = False):
    from concourse.bass_utils import run_bass_kernel_spmd

    yt = host_prep_y(np.asarray(y_pred, dtype=np.float32))
    oh = host_prep_oh(np.asarray(y_true))

    in_maps = []
    for i in range(NCORES):
        sl = slice(i * BS, (i + 1) * BS)
        in_maps.append({"yt": yt[sl], "oh": oh[sl]})

    nc = _get_nc()
    res = run_bass_kernel_spmd(nc, in_maps, list(range(NCORES)), trace=_trace)
    out = np.concatenate([res.results[i]["out"] for i in range(NCORES)], axis=0)
    if _trace:
        return out.astype(np.float32), res
    return out.astype(np.float32)


# revision 7
# speedup vs baseline: 1.0177x; 1.0177x over previous
"""CTC batch loss kernel for Trainium2 (8 NeuronCores, batch-parallel).

Math: reference computes logp = log_softmax(log(y+eps)) = log(y+eps) - log(rowsum),
then a log-space forward DP over the extended label sequence (S = 2L+1 = 129).
We run the DP in probability space with periodic renormalization:
  loss[b] = sum_t log(rowsum[b,t]) - sum_r log(c_r[b]) - log(tail[b])
where c_r are the renorm constants and tail is the (scaled) final alpha mass in
the two valid end states.

Per-core layout (32 samples/core):
  - y_pred is transposed on host to [b, tq, c(part), cchunk, t] so the
    per-sample gather matmul (contract over C) needs no on-chip transpose.
  - One-hot matrix O_b [1024, 130] per sample (host-built, bf16):
      lanes 0..63   = onehot(label_j)          -> u_label
      lanes 64..127 = skip_j * onehot(label_j) -> u_label pre-masked for skips
      lane  128     = onehot(blank=1023)       -> u_blank
      lane  129     = ones                     -> rowsum
  - PE: emis[t, lane] = sum_c Y^T[c, t] * O[c, lane], accumulated over 8 c-chunks,
    in t-quarters of 64 so the DP can start early.
  - ACT copies PSUM->SBUF (bf16), then an SBUF->SBUF DMA repacks per-sample
    [64t, 130] blocks into the DP layout emis[32b, 64t, 130lane].
  - DVE runs the sequential DP, 4 ops/step. With a_e = even(blank)-state and
    a_o = odd(label)-state alphas, substitute z(t) := a_e(t)/ub(t) (i.e.
    z(t) = a_e(t-1) + a_o(t-1)_sh, the pre-emission even sum). Then:
      z(t)   = z(t-1)*ub(t-1) + o(t-1)[j-1]     (scalar_tensor_tensor)
      w(t)   = z(t-1)*ub(t-1) + o(t-1)[j]       (scalar_tensor_tensor) = a_e+a_o
      XX     = [w | pad,o(t-1)] * [ul | su]     (one mult: B tile is laid out
                                                 [w(64) | 0 | o(64)] so the two
                                                 products fuse)
      o(t)   = XX[0:64] + XX[64:128]
  - Renorm by max(z, o) every 32 steps; log-domain corrections at the end.
"""

import os
import sys
from contextlib import ExitStack

import numpy as np

sys.path.insert(0, "/opt/trn_rl_repo")
sys.path.insert(0, "/root/.axon_site/_ro/trn_rl_repo")

import ml_dtypes  # noqa: E402

B, T, C, L = 256, 256, 1024, 64
NCORES = 8
BS = B // NCORES  # 32 samples per core
NLANE = 130  # 64 ul | 64 su | blank | ones
KCH = C // 128  # 8 contraction chunks
NQ = 4  # t quarters
TQW = T // NQ  # 64
NORM_EVERY = 32
NNORM = T // NORM_EVERY  # 8
BLANK = C - 1


# ---------------------------------------------------------------- host prep

def host_prep_y(y_pred: np.ndarray) -> np.ndarray:
    """[B, T, C] f32 -> [B, NQ, 128(c part), KCH, TQW(t)] f32 contiguous."""
    yt = y_pred.reshape(B, NQ, TQW, KCH, 128).transpose(0, 1, 4, 3, 2)
    return np.ascontiguousarray(yt)


def host_prep_oh(y_true: np.ndarray) -> np.ndarray:
    """[B, L] int -> one-hot+aux matrix [B, 128(c part), KCH, NLANE] bf16."""
    lab = y_true.astype(np.int64)  # [B, L]
    oh = np.zeros((B, C, NLANE), dtype=np.float32)
    bidx = np.arange(B)[:, None]
    jidx = np.arange(L)[None, :]
    # u_label lanes
    oh[bidx, lab, jidx] = 1.0
    # su lanes: skip_j = (j>=1) & (lab_j != lab_{j-1})
    skip = np.zeros((B, L), dtype=np.float32)
    skip[:, 1:] = (lab[:, 1:] != lab[:, :-1]).astype(np.float32)
    oh[bidx, lab, jidx + L] = skip
    # blank lane
    oh[:, BLANK, 2 * L] = 1.0
    # ones lane (rowsum)
    oh[:, :, 2 * L + 1] = 1.0
    # [B, C, NLANE] -> [B, KCH, 128, NLANE] -> [B, 128, KCH, NLANE]
    oh = oh.reshape(B, KCH, 128, NLANE).transpose(0, 2, 1, 3)
    return np.ascontiguousarray(oh).astype(ml_dtypes.bfloat16)


# ---------------------------------------------------------------- bass build

def build_nc():
    import concourse.bass as bass
    import concourse.tile as tile
    from concourse import bacc, mybir

    f32 = mybir.dt.float32
    bf16 = mybir.dt.bfloat16
    MULT = mybir.AluOpType.mult
    ADD = mybir.AluOpType.add

    nc = bacc.Bacc(None, target_bir_lowering=False)

    yt_d = nc.declare_dram_parameter("yt", [BS, NQ, 128, KCH, TQW], f32, isOutput=False)
    oh_d = nc.declare_dram_parameter("oh", [BS, 128, KCH, NLANE], bf16, isOutput=False)
    out_d = nc.declare_dram_parameter("out", [BS, 1], f32, isOutput=True)

    with tile.TileContext(nc) as tc:
        with ExitStack() as ctx:
            ohp = ctx.enter_context(tc.tile_pool(name="ohp", bufs=1))
            yp = ctx.enter_context(tc.tile_pool(name="yp", bufs=10))
            psp = ctx.enter_context(
                tc.tile_pool(name="psp", bufs=4, space=bass.MemorySpace.PSUM)
            )
            stp = ctx.enter_context(tc.tile_pool(name="stp", bufs=4))
            emp = ctx.enter_context(tc.tile_pool(name="emp", bufs=2))
            alp = ctx.enter_context(tc.tile_pool(name="alp", bufs=1))
            dpp = ctx.enter_context(tc.tile_pool(name="dpp", bufs=3))
            fin = ctx.enter_context(tc.tile_pool(name="fin", bufs=1))

            # one-hot matrices: resident for the whole kernel
            oh_sb = []
            for b in range(BS):
                t_oh = ohp.tile([128, KCH, NLANE], bf16, tag=f"oh{b}", name=f"oh{b}")
                nc.sync.dma_start(t_oh[:], oh_d[b])
                oh_sb.append(t_oh)

            # persistent DP state, double buffered:
            # Z[i]: [32, 65] pre-emission even sums (a_e = z * ub implicit)
            # Bt[i]: [32, 129] = [ w(64) | zero pad | o(64) ]
            Z = [alp.tile([BS, L + 1], f32, tag=f"z{i}", name=f"z{i}") for i in range(2)]
            Bt = [alp.tile([BS, 2 * L + 1], f32, tag=f"b{i}", name=f"b{i}") for i in range(2)]
            URS = fin.tile([BS, T, 2], f32)  # [ub, rowsum] per t
            NRM = fin.tile([BS, NNORM], f32)  # renorm constants
            for t_ in Z + Bt:
                nc.vector.memset(t_[:], 0.0)

            def dp_step(t, em):
                """One DP step: read state (t-1)%2, write t%2."""
                tt = t % TQW
                cur, prv = t % 2, (t - 1) % 2
                zp, bp = Z[prv], Bt[prv]
                zn, bn = Z[cur], Bt[cur]
                ub_prev = URS[:, t - 1, 0:1]  # [32,1] f32 scalar AP
                # z(t) = z(t-1)*ub(t-1) + o(t-1)[j-1]   (o[j-1] = bp[64:129])
                nc.vector.scalar_tensor_tensor(
                    zn[:], zp[:], ub_prev, bp[:, L : 2 * L + 1], MULT, ADD
                )
                # w(t) = z(t-1)[0:64]*ub(t-1) + o(t-1)[j]  -> bp[0:64]
                nc.vector.scalar_tensor_tensor(
                    bp[:, 0:L], zp[:, 0:L], ub_prev, bp[:, L + 1 : 2 * L + 1], MULT, ADD
                )
                # XX = [w | pad,o] * [ul | su]
                xx = dpp.tile([BS, 2 * L], f32, tag="xx")
                nc.vector.tensor_mul(xx[:], bp[:, 0 : 2 * L], em[:, tt, 0 : 2 * L])
                # o(t) = XX[0:64] + XX[64:128]
                nc.vector.tensor_add(bn[:, L + 1 : 2 * L + 1], xx[:, 0:L], xx[:, L : 2 * L])
                if t % NORM_EVERY == NORM_EVERY - 1:
                    r = t // NORM_EVERY
                    m1 = dpp.tile([BS, 1], f32, tag="m1")
                    rc = dpp.tile([BS, 1], f32, tag="rc")
                    nc.vector.tensor_reduce(
                        m1[:], zn[:], mybir.AxisListType.X, mybir.AluOpType.max
                    )
                    nc.vector.tensor_reduce(
                        NRM[:, r : r + 1], bn[:, L + 1 : 2 * L + 1],
                        mybir.AxisListType.X, mybir.AluOpType.max,
                    )
                    nc.vector.tensor_max(NRM[:, r : r + 1], NRM[:, r : r + 1], m1[:])
                    nc.vector.reciprocal(rc[:], NRM[:, r : r + 1])
                    nc.vector.tensor_scalar_mul(zn[:], zn[:], rc[:])
                    nc.vector.tensor_scalar_mul(
                        bn[:, L + 1 : 2 * L + 1], bn[:, L + 1 : 2 * L + 1], rc[:]
                    )

            for q in range(NQ):
                em = emp.tile([BS, TQW, NLANE], bf16, tag="em")
                for b in range(BS):
                    ybf = yp.tile([128, KCH, TQW], bf16, tag="ybf")
                    nc.gpsimd.dma_start(ybf[:], yt_d[b, q])  # f32 -> bf16 cast DMA
                    ps = psp.tile([TQW, NLANE], f32, tag="ps")
                    for k in range(KCH):
                        nc.tensor.matmul(
                            ps[:],
                            ybf[:, k, :],
                            oh_sb[b][:, k, :],
                            start=(k == 0),
                            stop=(k == KCH - 1),
                        )
                    st = stp.tile([TQW, NLANE], bf16, tag="st")
                    nc.scalar.copy(st[:], ps[:])
                    # repack [64t, 130] -> emis[b, :, :] (one partition row)
                    nc.sync.dma_start(em[b : b + 1], st[:])

                # ub + rowsum lanes out (f32) before the em buffer is recycled
                nc.vector.tensor_copy(
                    URS[:, q * TQW : (q + 1) * TQW, :], em[:, :, 2 * L : 2 * L + 2]
                )

                if q == 0:
                    # init at t=0: a_e(0) = [ub(0), 0...] -> z(0) = [1, 0...]
                    #              a_o(0) = [ul(0)[0], 0...]
                    nc.vector.memset(Z[0][:, 0:1], 1.0)
                    nc.vector.tensor_copy(Bt[0][:, L + 1 : L + 2], em[:, 0, 0:1])
                    steps = range(1, TQW)
                else:
                    steps = range(q * TQW, (q + 1) * TQW)
                for t in steps:
                    dp_step(t, em)

            # ---- final: loss = sum_t log rs - sum_r log c_r - log(tail)
            last = (T - 1) % 2
            tail = fin.tile([BS, 1], f32)
            # tail = o(T-1)[63] + z(T-1)[64]*ub(T-1)
            nc.vector.scalar_tensor_tensor(
                tail[:], Z[last][:, L : L + 1], URS[:, T - 1, 0:1],
                Bt[last][:, 2 * L : 2 * L + 1], MULT, ADD,
            )
            ln_tail = fin.tile([BS, 1], f32)
            nc.scalar.activation(ln_tail[:], tail[:], mybir.ActivationFunctionType.Ln)
            scr_n = fin.tile([BS, NNORM], f32)
            acc_n = fin.tile([BS, 1], f32)
            nc.scalar.activation(
                scr_n[:], NRM[:], mybir.ActivationFunctionType.Ln, accum_out=acc_n[:]
            )
            scr_r = fin.tile([BS, T], f32)
            acc_r = fin.tile([BS, 1], f32)
            nc.scalar.activation(
                scr_r[:], URS[:, :, 1], mybir.ActivationFunctionType.Ln,
                accum_out=acc_r[:],
            )
            loss = fin.tile([BS, 1], f32)
            nc.vector.tensor_sub(loss[:], acc_r[:], acc_n[:])
            nc.vector.tensor_sub(loss[:], loss[:], ln_tail[:])
            nc.sync.dma_start(out_d[:], loss[:])

    nc.compile()
    return nc


_NC_CACHE = {}


def _get_nc():
    if "nc" not in _NC_CACHE:
        _NC_CACHE["nc"] = build_nc()
    return _NC_CACHE["nc"]


# ---------------------------------------------------------------- entrypoint

def kernel(y_true: np.ndarray, y_pred: np.ndarray, _trace: bool = False):
    from concourse.bass_utils import run_bass_kernel_spmd

    yt = host_prep_y(np.asarray(y_pred, dtype=np.float32))
    oh = host_prep_oh(np.asarray(y_true))

    in_maps = []
    for i in range(NCORES):
        sl = slice(i * BS, (i + 1) * BS)
        in_maps.append({"yt": yt[sl], "oh": oh[sl]})

    nc = _get_nc()
    res = run_bass_kernel_spmd(nc, in_maps, list(range(NCORES)), trace=_trace)
    out = np.concatenate([res.results[i]["out"] for i in range(NCORES)], axis=0)
    if _trace:
        return out.astype(np.float32), res
    return out.astype(np.float32)


# revision 8
# speedup vs baseline: 1.1244x; 1.1049x over previous
"""CTC batch loss kernel for Trainium2 (8 NeuronCores, batch-parallel).

Math: reference computes logp = log_softmax(log(y+eps)) = log(y+eps) - log(rowsum),
then a log-space forward DP over the extended label sequence (S = 2L+1 = 129).
We run the DP in probability space with periodic renormalization, split into a
FORWARD chain (alpha, t=1..TSTAR) and a BACKWARD chain (beta, t=255..TSTAR+1)
that meet at TSTAR:
  loglik = log(sum_s alpha_T*[s] * beta_T*[s]) + sum log(c_r) - sum_t log(rowsum)
  loss   = -loglik
The two chains are independent, so their DVE ops are interleaved to hide the
~58-cycle SBUF access latency each dependent op would otherwise serialize on.

Per-core layout (32 samples/core):
  - y_pred transposed on host to [b, tq, c(part), cchunk, t]; per-sample gather
    matmul (contract over C) -> emis[t, lane], no on-chip transpose.
  - One-hot matrix O_b [1024, 130] per sample (host, bf16): lanes
    [ul(64) | su(64) | blank | ones]; su = skip-masked ul; ones lane = rowsum.
  - PE accumulates over 8 c-chunks in t-quarters of 64; ACT copies PSUM->SBUF
    bf16; SBUF->SBUF DMA repacks [64t,130] into emis[32b, 64t, 130].
  - Forward (4 DVE ops/step), state z (= a_e pre-emission sum; a_e = z*ub) and
    B = [w(64) | 0 | o(64) | 0]:
      w  = z[0:64]*ub(t-1) + o           (scalar_tensor_tensor)
      z' = z*ub(t-1) + [0,o]             (stt, in place)
      XX = [w | 0,o] * [ul | su]         (one mult vs adjacent em lanes)
      o' = XX[0:64] + XX[64:128]
  - Backward (4 DVE ops/step), state be=beta_even(65), bo=beta_odd(64):
      G  = [bo*ul | _ | bo*su]           (one mult, broadcast-read bo)
      t2 = be[1:65]*ub(t) + G[66:130]    (= g_e[j+1] + h[j+1])
      be'= be*ub(t) + G[0:65]            (stt, in place)
      bo'= t2 + G[0:64]
  - Renorm by max every 32 steps per chain; all log corrections at the end.
"""

import os
import sys
from contextlib import ExitStack

import numpy as np

sys.path.insert(0, "/opt/trn_rl_repo")
sys.path.insert(0, "/root/.axon_site/_ro/trn_rl_repo")

import ml_dtypes  # noqa: E402

B, T, C, L = 256, 256, 1024, 64
NCORES = 8
BS = B // NCORES  # 32 samples per core
NLANE = 130  # 64 ul | 64 su | blank | ones
KCH = C // 128  # 8 contraction chunks
NQ = 4  # t quarters
TQW = T // NQ  # 64
NORM_EVERY = 32
TSTAR = 152  # fwd computes alpha(1..TSTAR); bwd computes beta via t=255..TSTAR+1
NNF = 4  # fwd renorms at t = 31, 63, 95, 127
NNB = 3  # bwd renorms at t = 224, 192, 160
BLANK = C - 1


# ---------------------------------------------------------------- host prep

def host_prep_y(y_pred: np.ndarray) -> np.ndarray:
    """[B, T, C] f32 -> [B, NQ, 128(c part), KCH, TQW(t)] f32 contiguous."""
    yt = y_pred.reshape(B, NQ, TQW, KCH, 128).transpose(0, 1, 4, 3, 2)
    return np.ascontiguousarray(yt)


def host_prep_oh(y_true: np.ndarray) -> np.ndarray:
    """[B, L] int -> one-hot+aux matrix [B, 128(c part), KCH, NLANE] bf16."""
    lab = y_true.astype(np.int64)  # [B, L]
    oh = np.zeros((B, C, NLANE), dtype=np.float32)
    bidx = np.arange(B)[:, None]
    jidx = np.arange(L)[None, :]
    oh[bidx, lab, jidx] = 1.0  # ul lanes
    skip = np.zeros((B, L), dtype=np.float32)
    skip[:, 1:] = (lab[:, 1:] != lab[:, :-1]).astype(np.float32)
    oh[bidx, lab, jidx + L] = skip  # su lanes
    oh[:, BLANK, 2 * L] = 1.0  # blank lane
    oh[:, :, 2 * L + 1] = 1.0  # ones lane (rowsum)
    oh = oh.reshape(B, KCH, 128, NLANE).transpose(0, 2, 1, 3)
    return np.ascontiguousarray(oh).astype(ml_dtypes.bfloat16)


# ---------------------------------------------------------------- bass build

def build_nc():
    import concourse.bass as bass
    import concourse.tile as tile
    from concourse import bacc, mybir

    f32 = mybir.dt.float32
    bf16 = mybir.dt.bfloat16
    MULT = mybir.AluOpType.mult
    ADD = mybir.AluOpType.add

    nc = bacc.Bacc(None, target_bir_lowering=False)

    yt_d = nc.declare_dram_parameter("yt", [BS, NQ, 128, KCH, TQW], f32, isOutput=False)
    oh_d = nc.declare_dram_parameter("oh", [BS, 128, KCH, NLANE], bf16, isOutput=False)
    out_d = nc.declare_dram_parameter("out", [BS, 1], f32, isOutput=True)

    with tile.TileContext(nc) as tc:
        with ExitStack() as ctx:
            ohp = ctx.enter_context(tc.tile_pool(name="ohp", bufs=1))
            yp = ctx.enter_context(tc.tile_pool(name="yp", bufs=10))
            psp = ctx.enter_context(
                tc.tile_pool(name="psp", bufs=4, space=bass.MemorySpace.PSUM)
            )
            stp = ctx.enter_context(tc.tile_pool(name="stp", bufs=4))
            emp = ctx.enter_context(tc.tile_pool(name="emp", bufs=1))
            alp = ctx.enter_context(tc.tile_pool(name="alp", bufs=1))
            fin = ctx.enter_context(tc.tile_pool(name="fin", bufs=1))

            oh_sb = [None] * BS

            # persistent DP state (single buffered; updates are in-place safe)
            ZF = alp.tile([BS, L + 1], bf16, name="zf")  # fwd z (65)
            BF = alp.tile([BS, 2 * L + 2], bf16, name="bf")  # [w|0|o|0] (130)
            XX = alp.tile([BS, 2 * L], bf16, name="xx")  # fwd products
            BE = alp.tile([BS, L + 1], bf16, name="be")  # beta even (65)
            BO = alp.tile([BS, L], bf16, name="bo")  # beta odd (64)
            G = alp.tile([BS, 2 * L + 2], bf16, name="g")  # [g_o|0|h|0] (130)
            T2 = alp.tile([BS, L], bf16, name="t2")
            URS = fin.tile([BS, T, 2], f32)  # [ub, rowsum] per t
            NRM = fin.tile([BS, NNF + NNB], f32)  # renorm constants
            TMPM = alp.tile([BS, 1], f32, name="tmpm")
            TMPR = alp.tile([BS, 1], f32, name="tmpr")

            for t_ in (ZF, BF, XX, BE, BO, G, T2):
                nc.vector.memset(t_[:], 0.0)
            nc.vector.memset(ZF[:, 0:1], 1.0)  # z(0) = [1,0..] (a_e(0) = ub(0))
            nc.vector.memset(BE[:, L : L + 1], 1.0)  # beta_e[64] = 1 (s = 128)
            nc.vector.memset(BO[:, L - 1 : L], 1.0)  # beta_o[63] = 1 (s = 127)

            em_sb = {}

            def produce(q, load_oh):
                em = emp.tile([BS, TQW, NLANE], bf16, tag=f"em{q}", name=f"em{q}")
                em_sb[q] = em
                for b in range(BS):
                    if load_oh:
                        t_oh = ohp.tile(
                            [128, KCH, NLANE], bf16, tag=f"oh{b}", name=f"oh{b}"
                        )
                        nc.sync.dma_start(t_oh[:], oh_d[b])
                        oh_sb[b] = t_oh
                    ybf = yp.tile([128, KCH, TQW], bf16, tag="ybf", name="ybf")
                    nc.gpsimd.dma_start(ybf[:], yt_d[b, q])  # f32->bf16 cast DMA
                    ps = psp.tile([TQW, NLANE], f32, tag="ps", name="ps")
                    for k in range(KCH):
                        nc.tensor.matmul(
                            ps[:], ybf[:, k, :], oh_sb[b][:, k, :],
                            start=(k == 0), stop=(k == KCH - 1),
                        )
                    st = stp.tile([TQW, NLANE], bf16, tag="st", name="st")
                    nc.scalar.copy(st[:], ps[:])
                    nc.sync.dma_start(em[b : b + 1], st[:])

            def extract(q):
                nc.vector.tensor_copy(
                    URS[:, q * TQW : (q + 1) * TQW, :],
                    em_sb[q][:, :, 2 * L : 2 * L + 2],
                )

            def renorm(a65, b64, r):
                nc.vector.tensor_reduce(
                    TMPM[:], a65[:], mybir.AxisListType.X, mybir.AluOpType.max
                )
                nc.vector.tensor_reduce(
                    NRM[:, r : r + 1], b64[:], mybir.AxisListType.X,
                    mybir.AluOpType.max,
                )
                nc.vector.tensor_max(NRM[:, r : r + 1], NRM[:, r : r + 1], TMPM[:])
                nc.vector.reciprocal(TMPR[:], NRM[:, r : r + 1])
                nc.vector.tensor_scalar_mul(a65[:], a65[:], TMPR[:])
                nc.vector.tensor_scalar_mul(b64[:], b64[:], TMPR[:])

            def fwd_step(t):
                em = em_sb[t // TQW]
                tt = t % TQW
                ub = URS[:, t - 1, 0:1]
                # w = z[0:64]*ub + o
                nc.vector.scalar_tensor_tensor(
                    BF[:, 0:L], ZF[:, 0:L], ub, BF[:, L + 1 : 2 * L + 1], MULT, ADD
                )
                # z' = z*ub + [0|o]   (in place)
                nc.vector.scalar_tensor_tensor(
                    ZF[:], ZF[:], ub, BF[:, L : 2 * L + 1], MULT, ADD
                )
                # XX = [w | 0,o] * [ul | su]
                nc.vector.tensor_mul(XX[:], BF[:, 0 : 2 * L], em[:, tt, 0 : 2 * L])
                # o' = XX[0:64] + XX[64:128]
                nc.vector.tensor_add(
                    BF[:, L + 1 : 2 * L + 1], XX[:, 0:L], XX[:, L : 2 * L]
                )
                if t % NORM_EVERY == NORM_EVERY - 1 and t < 128:
                    renorm(ZF, BF[:, L + 1 : 2 * L + 1], t // NORM_EVERY)

            def bwd_step(t):
                """beta_{t-1} from beta_t (consumes emissions at t)."""
                em = em_sb[t // TQW]
                tt = t % TQW
                ub = URS[:, t, 0:1]
                # G = [bo*ul | _ | bo*su]
                g2 = G[:, 0 : 2 * (L + 1)].rearrange(
                    "p (a b) -> p a b", a=2, b=L + 1
                )[:, :, 0:L]
                bo2 = BO[:, None, 0:L].broadcast_to([BS, 2, L])
                em2 = em[:, tt, 0 : 2 * L].rearrange("p (a b) -> p a b", a=2, b=L)
                nc.vector.tensor_mul(g2, bo2, em2)
                # t2 = be[1:65]*ub + h[j+1]
                nc.vector.scalar_tensor_tensor(
                    T2[:], BE[:, 1 : L + 1], ub, G[:, L + 2 : 2 * L + 2], MULT, ADD
                )
                # be' = be*ub + [g_o | 0]   (in place)
                nc.vector.scalar_tensor_tensor(
                    BE[:], BE[:], ub, G[:, 0 : L + 1], MULT, ADD
                )
                # bo' = t2 + g_o
                nc.vector.tensor_add(BO[:], T2[:], G[:, 0:L])
                bi = 255 - t  # backward step index (0-based)
                if bi % NORM_EVERY == NORM_EVERY - 1:
                    renorm(BE, BO, NNF + bi // NORM_EVERY)

            # ---- emission schedule ----
            produce(0, load_oh=True)
            extract(0)
            # init: o(0)[0] = ul(0)[0]
            nc.vector.tensor_copy(BF[:, L + 1 : L + 2], em_sb[0][:, 0, 0:1])
            for t in range(1, TQW):
                fwd_step(t)
            produce(1, load_oh=False)
            extract(1)
            for t in range(TQW, 78):
                fwd_step(t)
            produce(3, load_oh=False)
            extract(3)
            produce(2, load_oh=False)
            fwd_iter = iter(range(78, TSTAR + 1))
            bwd_iter = iter(range(255, TSTAR, -1))
            pair = 0
            while True:
                ft = next(fwd_iter, None)
                bt = next(bwd_iter, None)
                if ft is None and bt is None:
                    break
                if pair == 40:
                    extract(2)
                if ft is not None:
                    fwd_step(ft)
                if bt is not None:
                    bwd_step(bt)
                pair += 1

            # ---- merge at TSTAR:
            # L = ub(T*) * sum(z*be) + sum(o*bo)
            M1 = fin.tile([BS, L + 1], f32)
            M2 = fin.tile([BS, L], f32)
            R1 = fin.tile([BS, 1], f32)
            R2 = fin.tile([BS, 1], f32)
            LS = fin.tile([BS, 1], f32)
            nc.vector.tensor_mul(M1[:], ZF[:], BE[:])
            nc.vector.tensor_mul(M2[:], BF[:, L + 1 : 2 * L + 1], BO[:])
            nc.vector.tensor_reduce(
                R1[:], M1[:], mybir.AxisListType.X, mybir.AluOpType.add
            )
            nc.vector.tensor_reduce(
                R2[:], M2[:], mybir.AxisListType.X, mybir.AluOpType.add
            )
            nc.vector.scalar_tensor_tensor(
                LS[:], R1[:], URS[:, TSTAR, 0:1], R2[:], MULT, ADD
            )
            ln_ls = fin.tile([BS, 1], f32)
            nc.scalar.activation(ln_ls[:], LS[:], mybir.ActivationFunctionType.Ln)
            scr_n = fin.tile([BS, NNF + NNB], f32)
            acc_n = fin.tile([BS, 1], f32)
            nc.scalar.activation(
                scr_n[:], NRM[:], mybir.ActivationFunctionType.Ln, accum_out=acc_n[:]
            )
            scr_r = fin.tile([BS, T], f32)
            acc_r = fin.tile([BS, 1], f32)
            nc.scalar.activation(
                scr_r[:], URS[:, :, 1], mybir.ActivationFunctionType.Ln,
                accum_out=acc_r[:],
            )
            loss = fin.tile([BS, 1], f32)
            nc.vector.tensor_sub(loss[:], acc_r[:], acc_n[:])
            nc.vector.tensor_sub(loss[:], loss[:], ln_ls[:])
            nc.sync.dma_start(out_d[:], loss[:])

    nc.compile()
    return nc


_NC_CACHE = {}


def _get_nc():
    if "nc" not in _NC_CACHE:
        _NC_CACHE["nc"] = build_nc()
    return _NC_CACHE["nc"]


# ---------------------------------------------------------------- entrypoint

def kernel(y_true: np.ndarray, y_pred: np.ndarray, _trace: bool = False):
    from concourse.bass_utils import run_bass_kernel_spmd

    yt = host_prep_y(np.asarray(y_pred, dtype=np.float32))
    oh = host_prep_oh(np.asarray(y_true))

    in_maps = []
    for i in range(NCORES):
        sl = slice(i * BS, (i + 1) * BS)
        in_maps.append({"yt": yt[sl], "oh": oh[sl]})

    nc = _get_nc()
    res = run_bass_kernel_spmd(nc, in_maps, list(range(NCORES)), trace=_trace)
    out = np.concatenate([res.results[i]["out"] for i in range(NCORES)], axis=0)
    if _trace:
        return out.astype(np.float32), res
    return out.astype(np.float32)


# revision 14
# speedup vs baseline: 1.1978x; 1.0652x over previous
"""CTC batch loss kernel for Trainium2 (8 NeuronCores, batch-parallel).

Math: reference computes logp = log_softmax(log(y+eps)) = log(y+eps) - log(rowsum),
then a log-space forward DP over the extended label sequence (S = 2L+1 = 129).
We run the DP in probability space with periodic renormalization, split into a
FORWARD chain (alpha, t=1..TSTAR) and a BACKWARD chain (beta, t=255..TSTAR+1)
that meet at TSTAR. Emission lanes are pre-divided by u_blank(t) (the blank
emission), which turns the blank-state updates into pure adds; the division
cancels in the final log-correction:
  loss[b] = sum_t [log rs(t) - log ub(t)] - sum_r log c_r - log(sum alpha~*beta~)

Per-core layout (32 samples/core):
  - y_pred transposed on host to [b, tq, c(part), cchunk, t]; per-sample gather
    matmul (contract over C) -> emis[t, lane], no on-chip transpose.
  - One-hot matrix O_b [1024, 130] per sample (host, bf16): lanes
    [ul(64) | su(64) | blank | ones]; su = skip-masked ul; ones lane = rowsum.
  - PE accumulates over 8 c-chunks in t-quarters of 64; ACT copies PSUM->SBUF
    bf16; SBUF->SBUF DMA repacks [64t,130] into emis[32b, 64t, 130]; GPSIMD
    scales lanes 0:128 by 1/ub(t) in bulk per quarter.
  - All DP ops are plain bf16 tensor_tensor add/mul (DVE 2x mode); fwd and bwd
    steps interleave [f1,b1,f2,b2,f3,b3,f4,b4] so each dependent pair is >= 2
    slots apart and the ~58-cycle SBUF latency is hidden.
      fwd: E(65)=alpha_even/prefix-ub, B=[q(64)|0|o(64)|0]:
        f1: q = o + E[0:64];  f2: E += [0|o];
        f3: XX = [q|0,o]*[ul'|su'];  f4: o = XX[0:64]+XX[64:128]
      bwd: BE(65), BO(64), G=[g_o(64)|0|h(64)|0]:
        b1: G = [BO*ul' | BO*su'] (broadcast-read BO)
        b2: T2 = BE[1:65]+G[66:130];  b3: BE += G[0:65];  b4: BO = T2+G[0:64]
  - Renorm by max every 32 steps per chain (+1 late fwd renorm pre-merge).
"""

import os
import sys
from contextlib import ExitStack

import numpy as np

sys.path.insert(0, "/opt/trn_rl_repo")
sys.path.insert(0, "/root/.axon_site/_ro/trn_rl_repo")

import ml_dtypes  # noqa: E402

B, T, C, L = 256, 256, 1024, 64
NCORES = 8
BS = B // NCORES  # 32 samples per core
NLANE = 130  # 64 ul | 64 su | blank | ones
KCH = C // 128  # 8 contraction chunks
NQ = 4
TQW = T // NQ  # 64
NORM_EVERY = 16
TSTAR = 172  # fwd computes alpha(1..TSTAR); bwd beta via t=255..TSTAR+1
FWD_RENORMS = list(range(15, 172, 16)) + [171]
NNF = len(FWD_RENORMS)  # 11
NNB = 5  # bwd renorms at backward-step index 15,31,47,63,79
BLANK = C - 1


# ---------------------------------------------------------------- host prep

def host_prep_y(y_pred: np.ndarray) -> np.ndarray:
    """[B, T, C] f32 -> [B, NQ, 128(c part), KCH, TQW(t)] f32 contiguous."""
    yt = y_pred.reshape(B, NQ, TQW, KCH, 128).transpose(0, 1, 4, 3, 2)
    return np.ascontiguousarray(yt)


def host_prep_oh(y_true: np.ndarray) -> np.ndarray:
    """[B, L] int -> one-hot+aux matrix [B, 128(c part), KCH, NLANE] bf16."""
    lab = y_true.astype(np.int64)
    oh = np.zeros((B, C, NLANE), dtype=np.float32)
    bidx = np.arange(B)[:, None]
    jidx = np.arange(L)[None, :]
    oh[bidx, lab, jidx] = 1.0  # ul lanes
    skip = np.zeros((B, L), dtype=np.float32)
    skip[:, 1:] = (lab[:, 1:] != lab[:, :-1]).astype(np.float32)
    oh[bidx, lab, jidx + L] = skip  # su lanes
    oh[:, BLANK, 2 * L] = 1.0  # blank lane
    oh[:, :, 2 * L + 1] = 1.0  # ones lane (rowsum)
    oh = oh.reshape(B, KCH, 128, NLANE).transpose(0, 2, 1, 3)
    return np.ascontiguousarray(oh).astype(ml_dtypes.bfloat16)


# ---------------------------------------------------------------- bass build

def build_nc():
    import concourse.bass as bass
    import concourse.tile as tile
    from concourse import bacc, mybir

    f32 = mybir.dt.float32
    bf16 = mybir.dt.bfloat16

    nc = bacc.Bacc(None, target_bir_lowering=False)

    yt_d = nc.declare_dram_parameter("yt", [BS, NQ, 128, KCH, TQW], f32, isOutput=False)
    oh_d = nc.declare_dram_parameter("oh", [BS, 128, KCH, NLANE], bf16, isOutput=False)
    out_d = nc.declare_dram_parameter("out", [BS, 1], f32, isOutput=True)

    with tile.TileContext(nc) as tc:
        with ExitStack() as ctx:
            ohp = ctx.enter_context(tc.tile_pool(name="ohp", bufs=1))
            yp = ctx.enter_context(tc.tile_pool(name="yp", bufs=10))
            psp = ctx.enter_context(
                tc.tile_pool(name="psp", bufs=4, space=bass.MemorySpace.PSUM)
            )
            stp = ctx.enter_context(tc.tile_pool(name="stp", bufs=4))
            emp = ctx.enter_context(tc.tile_pool(name="emp", bufs=1))
            alp = ctx.enter_context(tc.tile_pool(name="alp", bufs=1))
            fin = ctx.enter_context(tc.tile_pool(name="fin", bufs=1))

            oh_sb = [None] * BS

            # persistent DP state (single buffered; updates are in-place safe)
            EF = alp.tile([BS, L + 1], bf16, name="ef")  # fwd even-tilde (65)
            BF = alp.tile([BS, 2 * L + 2], bf16, name="bf")  # [q|0|o|0] (130)
            XX = alp.tile([BS, 2 * L], bf16, name="xx")
            BE = alp.tile([BS, L + 1], bf16, name="be")  # beta even (65)
            BO = alp.tile([BS, L], bf16, name="bo")  # beta odd (64)
            G = alp.tile([BS, 2 * L + 2], bf16, name="g")  # [g_o|0|h|0] (130)
            T2 = alp.tile([BS, L], bf16, name="t2")
            UBT = fin.tile([BS, T], f32)  # raw ub per t
            RST = fin.tile([BS, T], f32)  # raw rowsum per t
            RCB = fin.tile([BS, T], f32)  # 1/ub
            NRM = fin.tile([BS, NNF + NNB], f32)
            TMPM = alp.tile([BS, 1], f32, name="tmpm")
            TMPR = alp.tile([BS, 1], f32, name="tmpr")

            for t_ in (EF, BF, XX, BE, BO, G, T2):
                nc.vector.memset(t_[:], 0.0)
            nc.vector.memset(EF[:, 0:1], 1.0)  # e~(0) = [1,0..]
            nc.vector.memset(BE[:, L : L + 1], 1.0)  # beta_e[64] = 1 (s=128)
            nc.vector.memset(BO[:, L - 1 : L], 1.0)  # beta_o[63] = 1 (s=127)

            em_sb = {}

            def produce(q, load_oh=False):
                em = emp.tile([BS, TQW, NLANE], bf16, tag=f"em{q}", name=f"em{q}")
                em_sb[q] = em
                for b in range(BS):
                    if load_oh:
                        t_oh = ohp.tile(
                            [128, KCH, NLANE], bf16, tag=f"oh{b}", name=f"oh{b}"
                        )
                        nc.sync.dma_start(t_oh[:], oh_d[b])
                        oh_sb[b] = t_oh
                    ybf = yp.tile([128, KCH, TQW], bf16, tag="ybf", name="ybf")
                    nc.gpsimd.dma_start(ybf[:], yt_d[b, q])  # f32->bf16 cast DMA
                    ps = psp.tile([TQW, NLANE], f32, tag="ps", name="ps")
                    for k in range(KCH):
                        nc.tensor.matmul(
                            ps[:], ybf[:, k, :], oh_sb[b][:, k, :],
                            start=(k == 0), stop=(k == KCH - 1),
                        )
                    st = stp.tile([TQW, NLANE], bf16, tag="st", name="st")
                    nc.scalar.copy(st[:], ps[:])
                    nc.sync.dma_start(em[b : b + 1], st[:])

            def prep(q):
                """Extract raw ub/rs lanes; compute 1/ub (contiguous)."""
                qr = slice(q * TQW, (q + 1) * TQW)
                em = em_sb[q]
                nc.vector.tensor_single_scalar(
                    UBT[:, qr], em[:, :, 2 * L], 1e-30, mybir.AluOpType.max
                )
                nc.vector.tensor_copy(RST[:, qr], em[:, :, 2 * L + 1])
                nc.vector.reciprocal(RCB[:, qr], UBT[:, qr])

            def renorm(a65, b64, r):
                nc.vector.tensor_reduce(
                    TMPM[:], a65[:], mybir.AxisListType.X, mybir.AluOpType.max
                )
                nc.vector.tensor_reduce(
                    NRM[:, r : r + 1], b64[:], mybir.AxisListType.X,
                    mybir.AluOpType.max,
                )
                nc.vector.tensor_max(NRM[:, r : r + 1], NRM[:, r : r + 1], TMPM[:])
                nc.vector.reciprocal(TMPR[:], NRM[:, r : r + 1])
                nc.vector.tensor_scalar_mul(a65[:], a65[:], TMPR[:])
                nc.vector.tensor_scalar_mul(b64[:], b64[:], TMPR[:])

            def femit(t, phase):
                """Forward step t, op index phase (0..3)."""
                em = em_sb[t // TQW]
                tt = t % TQW
                if phase == 0:  # q = o + E[0:64]
                    nc.vector.tensor_add(
                        BF[:, 0:L], BF[:, L + 1 : 2 * L + 1], EF[:, 0:L]
                    )
                elif phase == 1:  # E += [0|o]  (in place; before f4 rewrites o)
                    nc.vector.tensor_add(EF[:], EF[:], BF[:, L : 2 * L + 1])
                elif phase == 2:  # XX = ([q|0,o]/ub(t)) * [ul|su]
                    nc.vector.scalar_tensor_tensor(
                        XX[:], BF[:, 0 : 2 * L], RCB[:, t : t + 1],
                        em[:, tt, 0 : 2 * L],
                        mybir.AluOpType.mult, mybir.AluOpType.mult,
                    )
                else:  # o = XX[0:64] + XX[64:128]
                    nc.vector.tensor_add(
                        BF[:, L + 1 : 2 * L + 1], XX[:, 0:L], XX[:, L : 2 * L]
                    )
                    if t in FWD_RENORMS:
                        renorm(EF, BF[:, L + 1 : 2 * L + 1], FWD_RENORMS.index(t))

            def bemit(t, phase):
                """Backward step consuming emissions at t, op index phase."""
                em = em_sb[t // TQW]
                tt = t % TQW
                if phase == 0:  # G = [(BO/ub)*ul | (BO/ub)*su]
                    g2 = G[:, 0 : 2 * (L + 1)].rearrange(
                        "p (a b) -> p a b", a=2, b=L + 1
                    )[:, :, 0:L]
                    bo2 = BO[:, None, 0:L].broadcast_to([BS, 2, L])
                    em2 = em[:, tt, 0 : 2 * L].rearrange("p (a b) -> p a b", a=2, b=L)
                    nc.vector.scalar_tensor_tensor(
                        g2, bo2, RCB[:, t : t + 1], em2,
                        mybir.AluOpType.mult, mybir.AluOpType.mult,
                    )
                elif phase == 1:  # T2 = BE[1:65] + h[j+1]
                    nc.vector.tensor_add(
                        T2[:], BE[:, 1 : L + 1], G[:, L + 2 : 2 * L + 2]
                    )
                elif phase == 2:  # BE += [g_o|0]  (in place)
                    nc.vector.tensor_add(BE[:], BE[:], G[:, 0 : L + 1])
                else:  # BO = T2 + g_o
                    nc.vector.tensor_add(BO[:], T2[:], G[:, 0:L])
                    bi = 255 - t
                    if bi % NORM_EVERY == NORM_EVERY - 1:
                        renorm(BE, BO, NNF + bi // NORM_EVERY)

            def fwd_step(t):
                for ph in range(4):
                    femit(t, ph)

            # ---- emission schedule ----
            produce(0, load_oh=True)
            prep(0)
            # init: o~(0)[0] = ul(0)[0]/ub(0)
            nc.vector.tensor_scalar_mul(
                BF[:, L + 1 : L + 2], em_sb[0][:, 0, 0:1], RCB[:, 0:1]
            )
            for t in range(1, TQW):
                fwd_step(t)
            produce(1)
            prep(1)
            for t in range(TQW, 91):
                fwd_step(t)
            produce(3)
            prep(3)
            produce(2)
            fwd_list = list(range(91, TSTAR + 1))  # 82 steps
            bwd_list = list(range(255, TSTAR, -1))  # 83 steps
            np_pairs = max(len(fwd_list), len(bwd_list))
            for i in range(np_pairs):
                if i == 30:
                    prep(2)
                ft = fwd_list[i] if i < len(fwd_list) else None
                bt = bwd_list[i] if i < len(bwd_list) else None
                for ph in range(4):
                    if ft is not None:
                        femit(ft, ph)
                    if bt is not None:
                        bemit(bt, ph)

            # ---- merge at TSTAR: L~ = sum(E*BE) + sum(o*BO)
            M1 = fin.tile([BS, L + 1], f32)
            M2 = fin.tile([BS, L], f32)
            R1 = fin.tile([BS, 1], f32)
            LS = fin.tile([BS, 1], f32)
            nc.vector.tensor_mul(M1[:], EF[:], BE[:])
            nc.vector.tensor_mul(M2[:], BF[:, L + 1 : 2 * L + 1], BO[:])
            nc.vector.tensor_reduce(
                R1[:], M1[:], mybir.AxisListType.X, mybir.AluOpType.add
            )
            nc.vector.tensor_reduce(
                LS[:], M2[:], mybir.AxisListType.X, mybir.AluOpType.add
            )
            nc.vector.tensor_add(LS[:], LS[:], R1[:])
            ln_ls = fin.tile([BS, 1], f32)
            nc.scalar.activation(ln_ls[:], LS[:], mybir.ActivationFunctionType.Ln)
            scr_n = fin.tile([BS, NNF + NNB], f32)
            acc_n = fin.tile([BS, 1], f32)
            nc.scalar.activation(
                scr_n[:], NRM[:], mybir.ActivationFunctionType.Ln,
                scale=float(2.0 ** -16), accum_out=acc_n[:]
            )
            scr_r = fin.tile([BS, T], f32)
            acc_r = fin.tile([BS, 1], f32)
            nc.scalar.activation(
                scr_r[:], RST[:], mybir.ActivationFunctionType.Ln,
                accum_out=acc_r[:],
            )
            scr_u = fin.tile([BS, T], f32)
            acc_u = fin.tile([BS, 1], f32)
            nc.scalar.activation(
                scr_u[:], UBT[:], mybir.ActivationFunctionType.Ln,
                accum_out=acc_u[:],
            )
            # loss = (acc_r - acc_u) - acc_n - ln_ls
            loss = fin.tile([BS, 1], f32)
            nc.vector.tensor_sub(loss[:], acc_r[:], acc_u[:])
            nc.vector.tensor_sub(loss[:], loss[:], acc_n[:])
            nc.vector.tensor_sub(loss[:], loss[:], ln_ls[:])
            # acc_n used Ln(m * 2^-16); add back (NNF+NNB)*16*ln2
            import math
            nc.vector.tensor_single_scalar(
                loss[:], loss[:], float((NNF + NNB) * 16.0 * math.log(2.0)),
                mybir.AluOpType.subtract,
            )
            nc.sync.dma_start(out_d[:], loss[:])

    nc._dbg = {
        "EF": EF.name, "BF": BF.name, "XX": XX.name, "BE": BE.name,
        "BO": BO.name, "G": G.name, "T2": T2.name, "UBT": UBT.name,
        "RST": RST.name, "RCB": RCB.name, "NRM": NRM.name,
        "em": {q: em_sb[q].name for q in em_sb},
    }
    nc.compile()
    return nc


_NC_CACHE = {}


def _get_nc():
    if "nc" not in _NC_CACHE:
        _NC_CACHE["nc"] = build_nc()
    return _NC_CACHE["nc"]


# ---------------------------------------------------------------- entrypoint

def kernel(y_true: np.ndarray, y_pred: np.ndarray, _trace: bool = False):
    from concourse.bass_utils import run_bass_kernel_spmd

    yt = host_prep_y(np.asarray(y_pred, dtype=np.float32))
    oh = host_prep_oh(np.asarray(y_true))

    in_maps = []
    for i in range(NCORES):
        sl = slice(i * BS, (i + 1) * BS)
        in_maps.append({"yt": yt[sl], "oh": oh[sl]})

    nc = _get_nc()
    res = run_bass_kernel_spmd(nc, in_maps, list(range(NCORES)), trace=_trace)
    out = np.concatenate([res.results[i]["out"] for i in range(NCORES)], axis=0)
    if _trace:
        return out.astype(np.float32), res
    return out.astype(np.float32)
